# revision 60
# baseline (speedup 1.0000x reference)
"""Single-head attention (b=4, n=2048, d_model=1024, head=64) on 8 TRN2 cores.

Sharding: 2-way data parallel over batch x 2-way sequence parallel over
queries. Core c handles batch c//2, query half c%2; k/v are computed on-core
for the full 2048-row sequence (cheap projections, no collectives).

Per-core dataflow (all layouts chosen so no on-device transpose of x is
needed -- x arrives pre-transposed from the host as xT [d_model, seq]):

  stage A  [wq/8|wk] @ xT(own)   -> psum rows 0:64 = qT(lo),  rows 64:128 = kT(own)
  stage B  [wk|wv]   @ xT(other) -> psum rows 0:64 = kT(oth), rows 64:128 = vT(oth)
  stage C  [wv|wq/8] @ xT(own)   -> psum rows 0:64 = vT(own), rows 64:128 = qT(hi)

  scoresT[k,q] = kT.T @ qT  (K=64 contraction, 2 k-tiles packed in the PE
  array via row tile_position), exp on ACT (no row-max: scores are O(1)),
  out^T[65, q] = [v|1].T @ attnT accumulated over k-chunks -- row 64 gives
  the softmax denominator for free. Final PE transpose + per-row reciprocal.
"""

import sys

if "/opt/trn_rl_repo" not in sys.path:
    sys.path.insert(0, "/opt/trn_rl_repo")

import numpy as np
import ml_dtypes

import concourse.bass as bass  # noqa: F401  (engine classes referenced via nc)
from concourse import bacc
import concourse.mybir as mybir
import concourse.tile as tile
from concourse.bass import ts, ds
from concourse.bass_utils import run_bass_kernel_spmd

BF16 = mybir.dt.bfloat16
F32 = mybir.dt.float32
AFT = mybir.ActivationFunctionType
NPBF16 = ml_dtypes.bfloat16

B, N, D, H = 4, 2048, 1024, 64
NCORES = 8
NQ = N // 2       # query rows per core
NCHUNK = D // 128  # d_model chunks
KT = N // 128      # k tiles per core


def _build_nc():
    nc = bacc.Bacc("TRN2", target_bir_lowering=False, debug=False)

    xt_d = nc.dram_tensor("xt", [NCHUNK, 128, N], BF16, kind="ExternalInput")
    wqk_d = nc.dram_tensor("wqk", [128, NCHUNK, 128], BF16, kind="ExternalInput")
    wkv_d = nc.dram_tensor("wkv", [128, NCHUNK, 128], BF16, kind="ExternalInput")
    wvq_d = nc.dram_tensor("wvq", [128, NCHUNK, 128], BF16, kind="ExternalInput")
    bias_d = nc.dram_tensor("bias", [128, 3], F32, kind="ExternalInput")
    idv_d = nc.dram_tensor("idv", [128, 128], BF16, kind="ExternalInput")
    id65_d = nc.dram_tensor("id65", [H + 1, H + 1], F32, kind="ExternalInput")
    out_d = nc.dram_tensor("out", [128, NQ // 128, H], F32, kind="ExternalOutput")

    with tile.TileContext(nc) as tc:
        with (
            tc.tile_pool(name="const", bufs=1) as cpool,
            tc.tile_pool(name="xt", bufs=6) as xpool,
            tc.tile_pool(name="main", bufs=1) as mpool,
            tc.tile_pool(name="attn", bufs=2) as apool,
            tc.tile_pool(name="small", bufs=2) as spool,
        ):
            # weights on the scalar HWDGE queue (ahead of odd x chunks); the
            # late-needed consts ride the sync queue behind the even x chunks
            wqk = cpool.tile([128, NCHUNK, 128], BF16)
            nc.scalar.dma_start(out=wqk, in_=wqk_d.ap())
            wkv = cpool.tile([128, NCHUNK, 128], BF16)
            nc.scalar.dma_start(out=wkv, in_=wkv_d.ap())
            # wvq rides the sync queue (needed ~5us later than wqk) to
            # balance the two HWDGE rings' transfer load; its DMA is emitted
            # after chunk 0's so it doesn't delay the first real matmuls
            wvq = cpool.tile([128, NCHUNK, 128], BF16)

            # lo/hi halves live in separate tiles so the ACT- and DVE-side
            # psum->sbuf copies carry no false same-tile dependencies
            qTlo = mpool.tile([128, NQ], BF16)
            qThi = mpool.tile([128, NQ], BF16)
            kTlo = mpool.tile([128, NQ], BF16)
            kThi = mpool.tile([128, NQ], BF16)
            vT = mpool.tile([128, NQ], BF16)
            wu = mpool.tile([128, 512], BF16)
            nc.vector.memset(wu[:], 1.0)
            vaug = mpool.tile([128, KT, H + 1], BF16)
            nc.vector.memset(vaug[:], 1.0)

            # ---- phase 1: projections (accumulate over d_model chunks) ----
            with tc.tile_pool(name="psum1", bufs=1, space="PSUM") as pp1:
                # allocation order fixes psum banks: psC 0-1, psA 2-3,
                # psB 4-5 -- chosen so phase-2 slots overlap the psum tensor
                # whose copies finish around when that slot is first needed
                psC = pp1.tile([128, NQ], F32)
                psA = pp1.tile([128, NQ], F32)
                psB = pp1.tile([128, NQ], F32)
                # warm-up burst: ~3.5us of back-to-back matmuls on a dummy
                # tile while the first x chunks stream in, so the PE HAM
                # clock-gate reaches 8/8 before the real matmuls start
                wu_ps = pp1.tile([128, 512], F32)
                for _ in range(13):
                    nc.tensor.matmul(wu_ps[:], lhsT=wu[:, 0:128], rhs=wu[:],
                                     start=True, stop=True)
                for c in range(NCHUNK):
                    xtile = xpool.tile([128, N], BF16, tag="xtile")
                    dma_eng = nc.sync if c % 2 == 0 else nc.scalar
                    dma_eng.dma_start(out=xtile, in_=xt_d[c])
                    if c == 0:
                        nc.sync.dma_start(out=wvq, in_=wvq_d.ap())
                    st, sp = c == 0, c == NCHUNK - 1
                    for s in range(2):
                        own = ds(NQ + s * 512, 512)
                        nc.tensor.matmul(psA[:, ds(s * 512, 512)], lhsT=wqk[:, c, :],
                                         rhs=xtile[:, own], start=st, stop=sp)
                    for s in range(2):
                        oth = ds(s * 512, 512)
                        nc.tensor.matmul(psB[:, ds(s * 512, 512)], lhsT=wkv[:, c, :],
                                         rhs=xtile[:, oth], start=st, stop=sp)
                    for s in range(2):
                        own = ds(NQ + s * 512, 512)
                        nc.tensor.matmul(psC[:, ds(s * 512, 512)], lhsT=wvq[:, c, :],
                                         rhs=xtile[:, own], start=st, stop=sp)

                bias_t = cpool.tile([128, 3], F32)
                nc.sync.dma_start(out=bias_t, in_=bias_d.ap())
                idv = cpool.tile([128, 128], BF16)
                nc.sync.dma_start(out=idv, in_=idv_d.ap())
                id65 = cpool.tile([H + 1, H + 1], F32)
                nc.sync.dma_start(out=id65, in_=id65_d.ap())

                # psum -> sbuf copies with fused bias add
                def cp_act(dst, src, bias_ap, sl):
                    nc.scalar.activation(out=dst[:, sl], in_=src[:, sl],
                                         func=AFT.Identity, bias=bias_ap)

                def cp_dve(dst, src, bias_ap, sl):
                    nc.vector.tensor_scalar_add(out=dst[:, sl], in0=src[:, sl],
                                                scalar1=bias_ap)

                qlo = (qTlo[0:64, :], psA[0:64, :], bias_t[0:64, 0:1])
                khi = (kThi[64:128, :], psA[64:128, :], bias_t[64:128, 0:1])
                klo = (kTlo[0:64, :], psB[0:64, :], bias_t[0:64, 1:2])
                vhi = (vT[64:128, :], psB[64:128, :], bias_t[64:128, 1:2])
                vlo = (vT[0:64, :], psC[0:64, :], bias_t[0:64, 2:3])
                qhi = (qThi[64:128, :], psC[64:128, :], bias_t[64:128, 2:3])
                # Tile's access tracking is tensor-granular, so cross-engine
                # readers of the same psum tensor serialize; the emission
                # order below sets the scheduler's cross-engine ordering so
                # the score-feeding copies land first on both engines
                sl = ds(0, NQ)
                cp_dve(*klo, sl)
                cp_act(*qlo, sl)
                cp_act(*khi, sl)
                cp_dve(*qhi, sl)
                cp_act(*vhi, sl)
                cp_dve(*vlo, sl)


            # ---- phase 2: v transpose, scores, softmax, attn @ v ----
            with tc.tile_pool(name="psum2", bufs=1, space="PSUM") as pp2:
                def v_trans_pair(p):
                    # one K=128 transpose of the full-partition vT column block
                    # yields v-natural for BOTH k-tiles p+8 (cols 0:64, from
                    # vT rows 0:64) and p (cols 64:128, from vT rows 64:128)
                    tp = pp2.tile([128, 128], BF16, tag="scores", bufs=3)
                    nc.tensor.transpose(out=tp[:], in_=vT[:, ts(p, 128)],
                                        identity=idv[:])
                    nc.vector.tensor_copy(out=vaug[:, p + 8, 0:H], in_=tp[:, 0:H])
                    nc.vector.tensor_copy(out=vaug[:, p, 0:H], in_=tp[:, H:2 * H])

                # two separate accumulators (one per 512-col half) so the
                # epilogue psum->sbuf copies can run on ACT and DVE in parallel
                outTa = pp2.tile([H + 1, 512], F32, tag="outTa")
                outTb = pp2.tile([H + 1, 512], F32, tag="outTb")
                outT_half = (outTa, outTb)

                def scores_pair(p):
                    # both row strips of the PE array run concurrently
                    sA = pp2.tile([128, NQ], F32, tag="scores", bufs=3)
                    sB = pp2.tile([128, NQ], F32, tag="scores", bufs=3)
                    for s in range(2):
                        nc.tensor.matmul(sA[:, ds(s * 512, 512)],
                                         lhsT=kTlo[0:64, ts(p, 128)],
                                         rhs=qTlo[0:64, ds(s * 512, 512)],
                                         start=True, stop=True, tile_position=(0, 0))
                        nc.tensor.matmul(sB[:, ds(s * 512, 512)],
                                         lhsT=kThi[64:128, ts(p, 128)],
                                         rhs=qThi[64:128, ds(s * 512, 512)],
                                         start=True, stop=True, tile_position=(64, 0))
                    return sA, sB

                def exp_pair(p, sA, sB):
                    atA = apool.tile([128, NQ], BF16, tag="at", bufs=4)
                    atB = apool.tile([128, NQ], BF16, tag="at", bufs=4)
                    nc.scalar.activation(out=atA[:], in_=sA[:], func=AFT.Exp)
                    nc.scalar.activation(out=atB[:], in_=sB[:], func=AFT.Exp)
                    return atA, atB

                def av_pair(p, atA, atB):
                    for half, at in ((0, atA), (1, atB)):
                        j = p + 8 * half
                        for s in range(2):
                            nc.tensor.matmul(outT_half[s][:],
                                             lhsT=vaug[:, j, :],
                                             rhs=at[:, ds(s * 512, 512)],
                                             start=(p == 0 and half == 0),
                                             stop=(p == 7 and half == 1))

                # software pipeline: emit pair p's scores before pair p-1's
                # attn@v so the PE never sits behind the exp on the ACT queue;
                # vaug[p] is produced inside iteration p, after the score
                # matmuls (it is first consumed by av_pair(p) one iteration
                # later). Exactly 3 "scores"-tag allocations per iteration
                # keeps the psum slot rotation stable.
                v_trans_pair(0)
                prev = None
                for p in range(8):
                    sA, sB = scores_pair(p)
                    ats = exp_pair(p, sA, sB)
                    if p < 7:
                        v_trans_pair(p + 1)
                    if prev is not None:
                        av_pair(p - 1, *prev)
                    prev = ats
                av_pair(7, *prev)

                # ---- epilogue: transpose out^T, divide by row sums, store ----
                # parallel psum->sbuf copies, then a 3-engine pipeline per
                # q-tile: PE transpose -> DVE reciprocal -> ACT scale
                outT_sba = mpool.tile([H + 1, 512], F32)
                outT_sbb = mpool.tile([H + 1, 512], F32)
                nc.scalar.activation(out=outT_sba[:], in_=outTa[:], func=AFT.Copy)
                nc.vector.tensor_copy(out=outT_sbb[:], in_=outTb[:])
                final = mpool.tile([128, NQ // 128, H], F32)
                for t in range(NQ // 128):
                    src_sb = outT_sba if t < 4 else outT_sbb
                    tp2 = pp2.tile([128, H + 1], F32, tag="scores", bufs=3)
                    nc.tensor.transpose(out=tp2[:], in_=src_sb[:, ts(t % 4, 128)],
                                        identity=id65[:])
                    rc = spool.tile([128, 1], F32, tag="rc", bufs=4)
                    nc.vector.reciprocal(out=rc[:], in_=tp2[:, H:H + 1])
                    if t % 2 == 0:
                        nc.scalar.activation(out=final[:, t, :], in_=tp2[:, 0:H],
                                             func=AFT.Copy, scale=rc[:])
                    else:
                        nc.vector.tensor_scalar_mul(out=final[:, t, :],
                                                    in0=tp2[:, 0:H],
                                                    scalar1=rc[:])
                    if t == 3:
                        nc.sync.dma_start(out=out_d.ap()[:, 0:4, :],
                                          in_=final[:, 0:4, :])
                nc.sync.dma_start(out=out_d.ap()[:, 4:8, :], in_=final[:, 4:8, :])

    nc.compile()
    return nc


_NC_CACHE = None


def _get_nc():
    global _NC_CACHE
    if _NC_CACHE is None:
        _NC_CACHE = _build_nc()
    return _NC_CACHE


def _make_in_maps(x, wq, bq, wk, bk, wv, bv):
    x = np.asarray(x, np.float32)
    wq = np.asarray(wq, np.float32)
    bq = np.asarray(bq, np.float32)
    wk = np.asarray(wk, np.float32)
    bk = np.asarray(bk, np.float32)
    wv = np.asarray(wv, np.float32)
    bv = np.asarray(bv, np.float32)

    wqs, bqs = wq / 8.0, bq / 8.0  # fold 1/sqrt(head) into q
    pack = lambda w: np.ascontiguousarray(
        w.reshape(NCHUNK, 128, 128).transpose(1, 0, 2)).astype(NPBF16)
    shared = {
        "wqk": pack(np.concatenate([wqs, wk], 1)),
        "wkv": pack(np.concatenate([wk, wv], 1)),
        "wvq": pack(np.concatenate([wv, wqs], 1)),
        "bias": np.ascontiguousarray(np.stack(
            [np.concatenate([bqs, bk]),
             np.concatenate([bk, bv]),
             np.concatenate([bv, bqs])], 1)).astype(np.float32),
        "idv": np.eye(128).astype(NPBF16),
        "id65": np.eye(H + 1, dtype=np.float32),
    }
    in_maps = []
    for c in range(NCORES):
        b, h = c // 2, c % 2
        own = x[b, h * NQ:(h + 1) * NQ]
        oth = x[b, (1 - h) * NQ:(2 - h) * NQ]
        xperm_t = np.concatenate([oth, own], 0).T  # [D, N], k order [other|own]
        xt = np.ascontiguousarray(
            xperm_t.reshape(NCHUNK, 128, N)).astype(NPBF16)
        in_maps.append({"xt": xt, **shared})
    return in_maps


def _gather(results):
    out = np.empty((B, N, H), np.float32)
    for c in range(NCORES):
        b, h = c // 2, c % 2
        o = np.asarray(results[c]["out"])  # [128, NQ//128, H]
        out[b, h * NQ:(h + 1) * NQ] = o.transpose(1, 0, 2).reshape(NQ, H)
    return out


def run(inputs, trace=False, tmpdir=None):
    nc = _get_nc()
    in_maps = _make_in_maps(**inputs)
    res = run_bass_kernel_spmd(nc, in_maps, list(range(NCORES)), trace=trace,
                               tmpdir=tmpdir)
    return _gather(res.results), res


def kernel(**inputs):
    out, _ = run(inputs, trace=False)
    return out


# revision 61
# speedup vs baseline: 1.0070x; 1.0070x over previous
"""Single-head attention (b=4, n=2048, d_model=1024, head=64) on 8 TRN2 cores.

Sharding: 2-way data parallel over batch x 2-way sequence parallel over
queries. Core c handles batch c//2, query half c%2; k/v are computed on-core
for the full 2048-row sequence (cheap projections, no collectives).

Per-core dataflow (all layouts chosen so no on-device transpose of x is
needed -- x arrives pre-transposed from the host as xT [d_model, seq]):

  stage A  [wq/8|wk] @ xT(own)   -> psum rows 0:64 = qT(lo),  rows 64:128 = kT(own)
  stage B  [wk|wv]   @ xT(other) -> psum rows 0:64 = kT(oth), rows 64:128 = vT(oth)
  stage C  [wv|wq/8] @ xT(own)   -> psum rows 0:64 = vT(own), rows 64:128 = qT(hi)

  scoresT[k,q] = kT.T @ qT  (K=64 contraction, 2 k-tiles packed in the PE
  array via row tile_position), exp on ACT (no row-max: scores are O(1)),
  out^T[65, q] = [v|1].T @ attnT accumulated over k-chunks -- row 64 gives
  the softmax denominator for free. Final PE transpose + per-row reciprocal.
"""

import sys

if "/opt/trn_rl_repo" not in sys.path:
    sys.path.insert(0, "/opt/trn_rl_repo")

import numpy as np
import ml_dtypes

import concourse.bass as bass  # noqa: F401  (engine classes referenced via nc)
from concourse import bacc
import concourse.mybir as mybir
import concourse.tile as tile
from concourse.bass import ts, ds
from concourse.bass_utils import run_bass_kernel_spmd

BF16 = mybir.dt.bfloat16
F32 = mybir.dt.float32
AFT = mybir.ActivationFunctionType
NPBF16 = ml_dtypes.bfloat16

B, N, D, H = 4, 2048, 1024, 64
NCORES = 8
NQ = N // 2       # query rows per core
NCHUNK = D // 128  # d_model chunks
KT = N // 128      # k tiles per core


def _build_nc():
    nc = bacc.Bacc("TRN2", target_bir_lowering=False, debug=False)

    xt_d = nc.dram_tensor("xt", [NCHUNK, 128, N], BF16, kind="ExternalInput")
    wqk_d = nc.dram_tensor("wqk", [128, NCHUNK, 128], BF16, kind="ExternalInput")
    wkv_d = nc.dram_tensor("wkv", [128, NCHUNK, 128], BF16, kind="ExternalInput")
    wvq_d = nc.dram_tensor("wvq", [128, NCHUNK, 128], BF16, kind="ExternalInput")
    bias_d = nc.dram_tensor("bias", [128, 3], F32, kind="ExternalInput")
    idv_d = nc.dram_tensor("idv", [128, 128], BF16, kind="ExternalInput")
    id65_d = nc.dram_tensor("id65", [H + 1, H + 1], F32, kind="ExternalInput")
    out_d = nc.dram_tensor("out", [128, NQ // 128, H], F32, kind="ExternalOutput")

    with tile.TileContext(nc) as tc:
        with (
            tc.tile_pool(name="const", bufs=1) as cpool,
            tc.tile_pool(name="xt", bufs=6) as xpool,
            tc.tile_pool(name="main", bufs=1) as mpool,
            tc.tile_pool(name="attn", bufs=2) as apool,
            tc.tile_pool(name="small", bufs=2) as spool,
        ):
            # weights on the scalar HWDGE queue (ahead of odd x chunks); the
            # late-needed consts ride the sync queue behind the even x chunks
            wqk = cpool.tile([128, NCHUNK, 128], BF16)
            nc.scalar.dma_start(out=wqk, in_=wqk_d.ap())
            wkv = cpool.tile([128, NCHUNK, 128], BF16)
            nc.scalar.dma_start(out=wkv, in_=wkv_d.ap())
            # wvq rides the sync queue (needed ~5us later than wqk) to
            # balance the two HWDGE rings' transfer load
            wvq = cpool.tile([128, NCHUNK, 128], BF16)
            nc.sync.dma_start(out=wvq, in_=wvq_d.ap())

            # lo/hi halves live in separate tiles so the ACT- and DVE-side
            # psum->sbuf copies carry no false same-tile dependencies
            qTlo = mpool.tile([128, NQ], BF16)
            qThi = mpool.tile([128, NQ], BF16)
            kTlo = mpool.tile([128, NQ], BF16)
            kThi = mpool.tile([128, NQ], BF16)
            vT = mpool.tile([128, NQ], BF16)
            wu = mpool.tile([128, 512], BF16)
            nc.vector.memset(wu[:], 1.0)
            vaug = mpool.tile([128, KT, H + 1], BF16)
            nc.vector.memset(vaug[:], 1.0)

            # ---- phase 1: projections (accumulate over d_model chunks) ----
            with tc.tile_pool(name="psum1", bufs=1, space="PSUM") as pp1:
                # allocation order fixes psum banks: psC 0-1, psA 2-3,
                # psB 4-5 -- chosen so phase-2 slots overlap the psum tensor
                # whose copies finish around when that slot is first needed
                psC = pp1.tile([128, NQ], F32)
                psA = pp1.tile([128, NQ], F32)
                psB = pp1.tile([128, NQ], F32)
                # warm-up burst: ~3.5us of back-to-back matmuls on a dummy
                # tile while the first x chunks stream in, so the PE HAM
                # clock-gate reaches 8/8 before the real matmuls start
                wu_ps = pp1.tile([128, 512], F32)
                for _ in range(13):
                    nc.tensor.matmul(wu_ps[:], lhsT=wu[:, 0:128], rhs=wu[:],
                                     start=True, stop=True)
                for c in range(NCHUNK):
                    xtile = xpool.tile([128, N], BF16, tag="xtile")
                    dma_eng = nc.sync if c % 2 == 0 else nc.scalar
                    dma_eng.dma_start(out=xtile, in_=xt_d[c])
                    st, sp = c == 0, c == NCHUNK - 1
                    for s in range(2):
                        own = ds(NQ + s * 512, 512)
                        nc.tensor.matmul(psA[:, ds(s * 512, 512)], lhsT=wqk[:, c, :],
                                         rhs=xtile[:, own], start=st, stop=sp)
                    for s in range(2):
                        oth = ds(s * 512, 512)
                        nc.tensor.matmul(psB[:, ds(s * 512, 512)], lhsT=wkv[:, c, :],
                                         rhs=xtile[:, oth], start=st, stop=sp)
                    for s in range(2):
                        own = ds(NQ + s * 512, 512)
                        nc.tensor.matmul(psC[:, ds(s * 512, 512)], lhsT=wvq[:, c, :],
                                         rhs=xtile[:, own], start=st, stop=sp)

                bias_t = cpool.tile([128, 3], F32)
                nc.sync.dma_start(out=bias_t, in_=bias_d.ap())
                idv = cpool.tile([128, 128], BF16)
                nc.sync.dma_start(out=idv, in_=idv_d.ap())
                id65 = cpool.tile([H + 1, H + 1], F32)
                nc.sync.dma_start(out=id65, in_=id65_d.ap())

                # psum -> sbuf copies with fused bias add
                def cp_act(dst, src, bias_ap, sl):
                    nc.scalar.activation(out=dst[:, sl], in_=src[:, sl],
                                         func=AFT.Identity, bias=bias_ap)

                def cp_dve(dst, src, bias_ap, sl):
                    nc.vector.tensor_scalar_add(out=dst[:, sl], in0=src[:, sl],
                                                scalar1=bias_ap)

                qlo = (qTlo[0:64, :], psA[0:64, :], bias_t[0:64, 0:1])
                khi = (kThi[64:128, :], psA[64:128, :], bias_t[64:128, 0:1])
                klo = (kTlo[0:64, :], psB[0:64, :], bias_t[0:64, 1:2])
                vhi = (vT[64:128, :], psB[64:128, :], bias_t[64:128, 1:2])
                vlo = (vT[0:64, :], psC[0:64, :], bias_t[0:64, 2:3])
                qhi = (qThi[64:128, :], psC[64:128, :], bias_t[64:128, 2:3])
                # Tile's access tracking is tensor-granular, so cross-engine
                # readers of the same psum tensor serialize; the emission
                # order below sets the scheduler's cross-engine ordering so
                # the score-feeding copies land first on both engines
                sl = ds(0, NQ)
                cp_dve(*klo, sl)
                cp_act(*qlo, sl)
                cp_act(*khi, sl)
                cp_dve(*qhi, sl)
                cp_act(*vhi, sl)
                cp_dve(*vlo, sl)


            # ---- phase 2: v transpose, scores, softmax, attn @ v ----
            with tc.tile_pool(name="psum2", bufs=1, space="PSUM") as pp2:
                def v_trans_pair(p):
                    # one K=128 transpose of the full-partition vT column block
                    # yields v-natural for BOTH k-tiles p+8 (cols 0:64, from
                    # vT rows 0:64) and p (cols 64:128, from vT rows 64:128)
                    tp = pp2.tile([128, 128], BF16, tag="scores", bufs=3)
                    nc.tensor.transpose(out=tp[:], in_=vT[:, ts(p, 128)],
                                        identity=idv[:])
                    nc.vector.tensor_copy(out=vaug[:, p + 8, 0:H], in_=tp[:, 0:H])
                    nc.vector.tensor_copy(out=vaug[:, p, 0:H], in_=tp[:, H:2 * H])

                # two separate accumulators (one per 512-col half) so the
                # epilogue psum->sbuf copies can run on ACT and DVE in parallel
                outTa = pp2.tile([H + 1, 512], F32, tag="outTa")
                outTb = pp2.tile([H + 1, 512], F32, tag="outTb")
                outT_half = (outTa, outTb)

                def scores_pair(p):
                    # both row strips of the PE array run concurrently
                    sA = pp2.tile([128, NQ], F32, tag="scores", bufs=3)
                    sB = pp2.tile([128, NQ], F32, tag="scores", bufs=3)
                    for s in range(2):
                        nc.tensor.matmul(sA[:, ds(s * 512, 512)],
                                         lhsT=kTlo[0:64, ts(p, 128)],
                                         rhs=qTlo[0:64, ds(s * 512, 512)],
                                         start=True, stop=True, tile_position=(0, 0))
                        nc.tensor.matmul(sB[:, ds(s * 512, 512)],
                                         lhsT=kThi[64:128, ts(p, 128)],
                                         rhs=qThi[64:128, ds(s * 512, 512)],
                                         start=True, stop=True, tile_position=(64, 0))
                    return sA, sB

                def exp_pair(p, sA, sB):
                    atA = apool.tile([128, NQ], BF16, tag="at", bufs=4)
                    atB = apool.tile([128, NQ], BF16, tag="at", bufs=4)
                    nc.scalar.activation(out=atA[:], in_=sA[:], func=AFT.Exp)
                    nc.scalar.activation(out=atB[:], in_=sB[:], func=AFT.Exp)
                    return atA, atB

                def av_pair(p, atA, atB):
                    for half, at in ((0, atA), (1, atB)):
                        j = p + 8 * half
                        for s in range(2):
                            nc.tensor.matmul(outT_half[s][:],
                                             lhsT=vaug[:, j, :],
                                             rhs=at[:, ds(s * 512, 512)],
                                             start=(p == 0 and half == 0),
                                             stop=(p == 7 and half == 1))

                # software pipeline: emit pair p's scores before pair p-1's
                # attn@v so the PE never sits behind the exp on the ACT queue;
                # vaug[p] is produced inside iteration p, after the score
                # matmuls (it is first consumed by av_pair(p) one iteration
                # later). Exactly 3 "scores"-tag allocations per iteration
                # keeps the psum slot rotation stable.
                v_trans_pair(0)
                prev = None
                for p in range(8):
                    sA, sB = scores_pair(p)
                    ats = exp_pair(p, sA, sB)
                    if p < 7:
                        v_trans_pair(p + 1)
                    if prev is not None:
                        av_pair(p - 1, *prev)
                    prev = ats
                av_pair(7, *prev)

                # ---- epilogue: transpose out^T, divide by row sums, store ----
                # parallel psum->sbuf copies, then a 3-engine pipeline per
                # q-tile: PE transpose -> DVE reciprocal -> ACT scale
                outT_sba = mpool.tile([H + 1, 512], F32)
                outT_sbb = mpool.tile([H + 1, 512], F32)
                nc.scalar.activation(out=outT_sba[:], in_=outTa[:], func=AFT.Copy)
                nc.vector.tensor_copy(out=outT_sbb[:], in_=outTb[:])
                final = mpool.tile([128, NQ // 128, H], F32)
                for t in range(NQ // 128):
                    src_sb = outT_sba if t < 4 else outT_sbb
                    tp2 = pp2.tile([128, H + 1], F32, tag="scores", bufs=3)
                    nc.tensor.transpose(out=tp2[:], in_=src_sb[:, ts(t % 4, 128)],
                                        identity=id65[:])
                    rc = spool.tile([128, 1], F32, tag="rc", bufs=4)
                    nc.vector.reciprocal(out=rc[:], in_=tp2[:, H:H + 1])
                    if t % 2 == 0:
                        nc.scalar.activation(out=final[:, t, :], in_=tp2[:, 0:H],
                                             func=AFT.Copy, scale=rc[:])
                    else:
                        nc.vector.tensor_scalar_mul(out=final[:, t, :],
                                                    in0=tp2[:, 0:H],
                                                    scalar1=rc[:])
                    if t == 3:
                        nc.sync.dma_start(out=out_d.ap()[:, 0:4, :],
                                          in_=final[:, 0:4, :])
                nc.sync.dma_start(out=out_d.ap()[:, 4:8, :], in_=final[:, 4:8, :])

    nc.compile()
    return nc


_NC_CACHE = None


def _get_nc():
    global _NC_CACHE
    if _NC_CACHE is None:
        _NC_CACHE = _build_nc()
    return _NC_CACHE


def _make_in_maps(x, wq, bq, wk, bk, wv, bv):
    x = np.asarray(x, np.float32)
    wq = np.asarray(wq, np.float32)
    bq = np.asarray(bq, np.float32)
    wk = np.asarray(wk, np.float32)
    bk = np.asarray(bk, np.float32)
    wv = np.asarray(wv, np.float32)
    bv = np.asarray(bv, np.float32)

    wqs, bqs = wq / 8.0, bq / 8.0  # fold 1/sqrt(head) into q
    pack = lambda w: np.ascontiguousarray(
        w.reshape(NCHUNK, 128, 128).transpose(1, 0, 2)).astype(NPBF16)
    shared = {
        "wqk": pack(np.concatenate([wqs, wk], 1)),
        "wkv": pack(np.concatenate([wk, wv], 1)),
        "wvq": pack(np.concatenate([wv, wqs], 1)),
        "bias": np.ascontiguousarray(np.stack(
            [np.concatenate([bqs, bk]),
             np.concatenate([bk, bv]),
             np.concatenate([bv, bqs])], 1)).astype(np.float32),
        "idv": np.eye(128).astype(NPBF16),
        "id65": np.eye(H + 1, dtype=np.float32),
    }
    in_maps = []
    for c in range(NCORES):
        b, h = c // 2, c % 2
        own = x[b, h * NQ:(h + 1) * NQ]
        oth = x[b, (1 - h) * NQ:(2 - h) * NQ]
        xperm_t = np.concatenate([oth, own], 0).T  # [D, N], k order [other|own]
        xt = np.ascontiguousarray(
            xperm_t.reshape(NCHUNK, 128, N)).astype(NPBF16)
        in_maps.append({"xt": xt, **shared})
    return in_maps


def _gather(results):
    out = np.empty((B, N, H), np.float32)
    for c in range(NCORES):
        b, h = c // 2, c % 2
        o = np.asarray(results[c]["out"])  # [128, NQ//128, H]
        out[b, h * NQ:(h + 1) * NQ] = o.transpose(1, 0, 2).reshape(NQ, H)
    return out


def run(inputs, trace=False, tmpdir=None):
    nc = _get_nc()
    in_maps = _make_in_maps(**inputs)
    res = run_bass_kernel_spmd(nc, in_maps, list(range(NCORES)), trace=trace,
                               tmpdir=tmpdir)
    return _gather(res.results), res


def kernel(**inputs):
    out, _ = run(inputs, trace=False)
    return out


# revision 62
# speedup vs baseline: 1.0648x; 1.0574x over previous
"""Single-head attention (b=4, n=2048, d_model=1024, head=64) on 8 TRN2 cores.

Sharding: 2-way data parallel over batch x 2-way sequence parallel over
queries. Core c handles batch c//2, query half c%2; k/v are computed on-core
for the full 2048-row sequence (cheap projections, no collectives).

Per-core dataflow (all layouts chosen so no on-device transpose of x is
needed -- x arrives pre-transposed from the host as xT [d_model, seq]):

  stage A  [wq/8|wk] @ xT(own)   -> psum rows 0:64 = qT(lo),  rows 64:128 = kT(own)
  stage B  [wk|wv]   @ xT(other) -> psum rows 0:64 = kT(oth), rows 64:128 = vT(oth)
  stage C  [wv|wq/8] @ xT(own)   -> psum rows 0:64 = vT(own), rows 64:128 = qT(hi)

  scoresT[k,q] = kT.T @ qT  (K=64 contraction, 2 k-tiles packed in the PE
  array via row tile_position), exp on ACT (no row-max: scores are O(1)),
  out^T[65, q] = [v|1].T @ attnT accumulated over k-chunks -- row 64 gives
  the softmax denominator for free. Final PE transpose + per-row reciprocal.
"""

import sys

if "/opt/trn_rl_repo" not in sys.path:
    sys.path.insert(0, "/opt/trn_rl_repo")

import numpy as np
import ml_dtypes

import concourse.bass as bass  # noqa: F401  (engine classes referenced via nc)
from concourse import bacc
import concourse.mybir as mybir
import concourse.tile as tile
from concourse.bass import ts, ds
from concourse.bass_utils import run_bass_kernel_spmd

BF16 = mybir.dt.bfloat16
F32 = mybir.dt.float32
AFT = mybir.ActivationFunctionType
NPBF16 = ml_dtypes.bfloat16

B, N, D, H = 4, 2048, 1024, 64
NCORES = 8
NQ = N // 2       # query rows per core
NCHUNK = D // 128  # d_model chunks
KT = N // 128      # k tiles per core


def _build_nc():
    nc = bacc.Bacc("TRN2", target_bir_lowering=False, debug=False)

    xt_d = nc.dram_tensor("xt", [NCHUNK, 128, N], BF16, kind="ExternalInput")
    wqk_d = nc.dram_tensor("wqk", [128, NCHUNK, 128], BF16, kind="ExternalInput")
    wkv_d = nc.dram_tensor("wkv", [128, NCHUNK, 128], BF16, kind="ExternalInput")
    wvq_d = nc.dram_tensor("wvq", [128, NCHUNK, 128], BF16, kind="ExternalInput")
    bias_d = nc.dram_tensor("bias", [128, 3], F32, kind="ExternalInput")
    idv_d = nc.dram_tensor("idv", [128, 128], BF16, kind="ExternalInput")
    id65_d = nc.dram_tensor("id65", [H + 1, H + 1], F32, kind="ExternalInput")
    out_d = nc.dram_tensor("out", [128, NQ // 128, H], F32, kind="ExternalOutput")

    with tile.TileContext(nc) as tc:
        with (
            tc.tile_pool(name="const", bufs=1) as cpool,
            tc.tile_pool(name="xt", bufs=6) as xpool,
            tc.tile_pool(name="main", bufs=1) as mpool,
            tc.tile_pool(name="attn", bufs=2) as apool,
            tc.tile_pool(name="small", bufs=2) as spool,
        ):
            # weights on the scalar HWDGE queue (ahead of odd x chunks); the
            # late-needed consts ride the sync queue behind the even x chunks
            wqk = cpool.tile([128, NCHUNK, 128], BF16)
            nc.scalar.dma_start(out=wqk, in_=wqk_d.ap())
            wkv = cpool.tile([128, NCHUNK, 128], BF16)
            nc.scalar.dma_start(out=wkv, in_=wkv_d.ap())
            # wvq rides the sync queue (needed ~5us later than wqk) to
            # balance the two HWDGE rings' transfer load
            wvq = cpool.tile([128, NCHUNK, 128], BF16)
            nc.sync.dma_start(out=wvq, in_=wvq_d.ap())

            # lo/hi halves live in separate tiles so the ACT- and DVE-side
            # psum->sbuf copies carry no false same-tile dependencies
            qTlo = mpool.tile([128, NQ], BF16)
            qThi = mpool.tile([128, NQ], BF16)
            kTlo = mpool.tile([128, NQ], BF16)
            kThi = mpool.tile([128, NQ], BF16)
            vT = mpool.tile([128, NQ], BF16)
            wu = mpool.tile([128, 512], BF16)
            nc.vector.memset(wu[:], 1.0)
            vaug = mpool.tile([128, KT, H + 1], BF16)
            nc.vector.memset(vaug[:], 1.0)

            # ---- phase 1: projections (accumulate over d_model chunks) ----
            with tc.tile_pool(name="psum1", bufs=1, space="PSUM") as pp1:
                # allocation order fixes psum banks: psC 0-1, psA 2-3,
                # psB 4-5 -- chosen so phase-2 slots overlap the psum tensor
                # whose copies finish around when that slot is first needed
                psC = pp1.tile([128, NQ], F32)
                psA = pp1.tile([128, NQ], F32)
                psB = pp1.tile([128, NQ], F32)
                # warm-up burst: ~3.5us of back-to-back matmuls on a dummy
                # tile while the first x chunks stream in, so the PE HAM
                # clock-gate reaches 8/8 before the real matmuls start
                wu_ps = pp1.tile([128, 512], F32)
                for _ in range(17):
                    nc.tensor.matmul(wu_ps[:], lhsT=wu[:, 0:128], rhs=wu[:],
                                     start=True, stop=True)
                for c in range(NCHUNK):
                    xtile = xpool.tile([128, N], BF16, tag="xtile")
                    dma_eng = nc.sync if c % 2 == 0 else nc.scalar
                    dma_eng.dma_start(out=xtile, in_=xt_d[c])
                    st, sp = c == 0, c == NCHUNK - 1
                    for s in range(2):
                        own = ds(NQ + s * 512, 512)
                        nc.tensor.matmul(psA[:, ds(s * 512, 512)], lhsT=wqk[:, c, :],
                                         rhs=xtile[:, own], start=st, stop=sp)
                    for s in range(2):
                        oth = ds(s * 512, 512)
                        nc.tensor.matmul(psB[:, ds(s * 512, 512)], lhsT=wkv[:, c, :],
                                         rhs=xtile[:, oth], start=st, stop=sp)
                    for s in range(2):
                        own = ds(NQ + s * 512, 512)
                        nc.tensor.matmul(psC[:, ds(s * 512, 512)], lhsT=wvq[:, c, :],
                                         rhs=xtile[:, own], start=st, stop=sp)

                bias_t = cpool.tile([128, 3], F32)
                nc.sync.dma_start(out=bias_t, in_=bias_d.ap())
                idv = cpool.tile([128, 128], BF16)
                nc.sync.dma_start(out=idv, in_=idv_d.ap())
                id65 = cpool.tile([H + 1, H + 1], F32)
                nc.sync.dma_start(out=id65, in_=id65_d.ap())

                # psum -> sbuf copies with fused bias add
                def cp_act(dst, src, bias_ap, sl):
                    nc.scalar.activation(out=dst[:, sl], in_=src[:, sl],
                                         func=AFT.Identity, bias=bias_ap)

                def cp_dve(dst, src, bias_ap, sl):
                    nc.vector.tensor_scalar_add(out=dst[:, sl], in0=src[:, sl],
                                                scalar1=bias_ap)

                qlo = (qTlo[0:64, :], psA[0:64, :], bias_t[0:64, 0:1])
                khi = (kThi[64:128, :], psA[64:128, :], bias_t[64:128, 0:1])
                klo = (kTlo[0:64, :], psB[0:64, :], bias_t[0:64, 1:2])
                vhi = (vT[64:128, :], psB[64:128, :], bias_t[64:128, 1:2])
                vlo = (vT[0:64, :], psC[0:64, :], bias_t[0:64, 2:3])
                qhi = (qThi[64:128, :], psC[64:128, :], bias_t[64:128, 2:3])
                # Tile's access tracking is tensor-granular, so cross-engine
                # readers of the same psum tensor serialize; the emission
                # order below sets the scheduler's cross-engine ordering so
                # the score-feeding copies land first on both engines
                sl = ds(0, NQ)
                cp_dve(*klo, sl)
                cp_act(*qlo, sl)
                cp_act(*khi, sl)
                cp_dve(*qhi, sl)
                cp_act(*vhi, sl)
                cp_dve(*vlo, sl)


            # ---- phase 2: v transpose, scores, softmax, attn @ v ----
            with tc.tile_pool(name="psum2", bufs=1, space="PSUM") as pp2:
                def v_trans_pair(p):
                    # one K=128 transpose of the full-partition vT column block
                    # yields v-natural for BOTH k-tiles p+8 (cols 0:64, from
                    # vT rows 0:64) and p (cols 64:128, from vT rows 64:128)
                    tp = pp2.tile([128, 128], BF16, tag="scores", bufs=3)
                    nc.tensor.transpose(out=tp[:], in_=vT[:, ts(p, 128)],
                                        identity=idv[:])
                    nc.vector.tensor_copy(out=vaug[:, p + 8, 0:H], in_=tp[:, 0:H])
                    nc.vector.tensor_copy(out=vaug[:, p, 0:H], in_=tp[:, H:2 * H])

                # two separate accumulators (one per 512-col half) so the
                # epilogue psum->sbuf copies can run on ACT and DVE in parallel
                outTa = pp2.tile([H + 1, 512], F32, tag="outTa")
                outTb = pp2.tile([H + 1, 512], F32, tag="outTb")
                outT_half = (outTa, outTb)

                def scores_pair(p):
                    # both row strips of the PE array run concurrently
                    sA = pp2.tile([128, NQ], F32, tag="scores", bufs=3)
                    sB = pp2.tile([128, NQ], F32, tag="scores", bufs=3)
                    for s in range(2):
                        nc.tensor.matmul(sA[:, ds(s * 512, 512)],
                                         lhsT=kTlo[0:64, ts(p, 128)],
                                         rhs=qTlo[0:64, ds(s * 512, 512)],
                                         start=True, stop=True, tile_position=(0, 0))
                        nc.tensor.matmul(sB[:, ds(s * 512, 512)],
                                         lhsT=kThi[64:128, ts(p, 128)],
                                         rhs=qThi[64:128, ds(s * 512, 512)],
                                         start=True, stop=True, tile_position=(64, 0))
                    return sA, sB

                def exp_pair(p, sA, sB):
                    atA = apool.tile([128, NQ], BF16, tag="at", bufs=4)
                    atB = apool.tile([128, NQ], BF16, tag="at", bufs=4)
                    nc.scalar.activation(out=atA[:], in_=sA[:], func=AFT.Exp)
                    nc.scalar.activation(out=atB[:], in_=sB[:], func=AFT.Exp)
                    return atA, atB

                def av_pair(p, atA, atB):
                    for half, at in ((0, atA), (1, atB)):
                        j = p + 8 * half
                        for s in range(2):
                            nc.tensor.matmul(outT_half[s][:],
                                             lhsT=vaug[:, j, :],
                                             rhs=at[:, ds(s * 512, 512)],
                                             start=(p == 0 and half == 0),
                                             stop=(p == 7 and half == 1))

                # software pipeline: emit pair p's scores before pair p-1's
                # attn@v so the PE never sits behind the exp on the ACT queue;
                # vaug[p] is produced inside iteration p, after the score
                # matmuls (it is first consumed by av_pair(p) one iteration
                # later). Exactly 3 "scores"-tag allocations per iteration
                # keeps the psum slot rotation stable.
                v_trans_pair(0)
                prev = None
                for p in range(8):
                    sA, sB = scores_pair(p)
                    ats = exp_pair(p, sA, sB)
                    if p < 7:
                        v_trans_pair(p + 1)
                    if prev is not None:
                        av_pair(p - 1, *prev)
                    prev = ats
                av_pair(7, *prev)

                # ---- epilogue: transpose out^T, divide by row sums, store ----
                # parallel psum->sbuf copies, then a 3-engine pipeline per
                # q-tile: PE transpose -> DVE reciprocal -> ACT scale
                outT_sba = mpool.tile([H + 1, 512], F32)
                outT_sbb = mpool.tile([H + 1, 512], F32)
                nc.scalar.activation(out=outT_sba[:], in_=outTa[:], func=AFT.Copy)
                nc.vector.tensor_copy(out=outT_sbb[:], in_=outTb[:])
                final = mpool.tile([128, NQ // 128, H], F32)
                for t in range(NQ // 128):
                    src_sb = outT_sba if t < 4 else outT_sbb
                    tp2 = pp2.tile([128, H + 1], F32, tag="scores", bufs=3)
                    nc.tensor.transpose(out=tp2[:], in_=src_sb[:, ts(t % 4, 128)],
                                        identity=id65[:])
                    rc = spool.tile([128, 1], F32, tag="rc", bufs=4)
                    nc.vector.reciprocal(out=rc[:], in_=tp2[:, H:H + 1])
                    if t % 2 == 0:
                        nc.scalar.activation(out=final[:, t, :], in_=tp2[:, 0:H],
                                             func=AFT.Copy, scale=rc[:])
                    else:
                        nc.vector.tensor_scalar_mul(out=final[:, t, :],
                                                    in0=tp2[:, 0:H],
                                                    scalar1=rc[:])
                    if t == 3:
                        nc.sync.dma_start(out=out_d.ap()[:, 0:4, :],
                                          in_=final[:, 0:4, :])
                nc.sync.dma_start(out=out_d.ap()[:, 4:8, :], in_=final[:, 4:8, :])

    nc.compile()
    return nc


_NC_CACHE = None


def _get_nc():
    global _NC_CACHE
    if _NC_CACHE is None:
        _NC_CACHE = _build_nc()
    return _NC_CACHE


def _make_in_maps(x, wq, bq, wk, bk, wv, bv):
    x = np.asarray(x, np.float32)
    wq = np.asarray(wq, np.float32)
    bq = np.asarray(bq, np.float32)
    wk = np.asarray(wk, np.float32)
    bk = np.asarray(bk, np.float32)
    wv = np.asarray(wv, np.float32)
    bv = np.asarray(bv, np.float32)

    wqs, bqs = wq / 8.0, bq / 8.0  # fold 1/sqrt(head) into q
    pack = lambda w: np.ascontiguousarray(
        w.reshape(NCHUNK, 128, 128).transpose(1, 0, 2)).astype(NPBF16)
    shared = {
        "wqk": pack(np.concatenate([wqs, wk], 1)),
        "wkv": pack(np.concatenate([wk, wv], 1)),
        "wvq": pack(np.concatenate([wv, wqs], 1)),
        "bias": np.ascontiguousarray(np.stack(
            [np.concatenate([bqs, bk]),
             np.concatenate([bk, bv]),
             np.concatenate([bv, bqs])], 1)).astype(np.float32),
        "idv": np.eye(128).astype(NPBF16),
        "id65": np.eye(H + 1, dtype=np.float32),
    }
    in_maps = []
    for c in range(NCORES):
        b, h = c // 2, c % 2
        own = x[b, h * NQ:(h + 1) * NQ]
        oth = x[b, (1 - h) * NQ:(2 - h) * NQ]
        xperm_t = np.concatenate([oth, own], 0).T  # [D, N], k order [other|own]
        xt = np.ascontiguousarray(
            xperm_t.reshape(NCHUNK, 128, N)).astype(NPBF16)
        in_maps.append({"xt": xt, **shared})
    return in_maps


def _gather(results):
    out = np.empty((B, N, H), np.float32)
    for c in range(NCORES):
        b, h = c // 2, c % 2
        o = np.asarray(results[c]["out"])  # [128, NQ//128, H]
        out[b, h * NQ:(h + 1) * NQ] = o.transpose(1, 0, 2).reshape(NQ, H)
    return out


def run(inputs, trace=False, tmpdir=None):
    nc = _get_nc()
    in_maps = _make_in_maps(**inputs)
    res = run_bass_kernel_spmd(nc, in_maps, list(range(NCORES)), trace=trace,
                               tmpdir=tmpdir)
    return _gather(res.results), res


def kernel(**inputs):
    out, _ = run(inputs, trace=False)
    return out
